# revision 1
# baseline (speedup 1.0000x reference)
"""CRF forward (partition function) kernel for Trainium2, 8 NeuronCores.

Meet-in-the-middle formulation (exp space), data-parallel over batch:
  forward   F_{i+1} = ef_i * (W @ F_i),            i = 0..M-1   (alpha side)
  backward  G_t = W^T @ (ef_t * G_{t+1}) + 1[length==t] * exp(trans[END]),
run from both ends to the midpoint M = S/2 (lengths >= S/2, so the forward
half is mask-free); host combines out[b] = log(F_M . G_M) + accumulators.

W[next,prev] = exp(trans[next,prev]); ef is exp(feat - max_tag feat) (host
prescale, bookkept via cumsum); every NK steps the device renormalizes each
batch column by r ~ 1/colsum (computed on-device, applied to a later ef
slice, exact r values dumped for host compensation).

The backward injection rides inside the one matmul per step: the state is
augmented with 3 extra rows -- row 64 a self-perpetuating constant 1, rows
65/66 per-tag-group injection markers delivered via the ef stream (marker
row at time t = 1[length==t]); the stationary has columns that (a) copy the
constant row forward and (b) add exp(trans[END])[prev] * marker to each
group's state rows.  No extra instructions, no PSUM read-modify-write.

Layout per chain: 2 tag-groups of 32 tags stacked on partitions, 64 batch
elems on the free dim; one chain per direction (forward 64 partitions,
backward 67).  The serial critical path per chain step is the PE->DVE
semaphore round trip (~500ns); the two chains interleave on the engines.
"""

import os
import sys

import numpy as np
import ml_dtypes

if "/opt/trn_rl_repo" not in sys.path:
    sys.path.insert(0, "/opt/trn_rl_repo")

import concourse.bass as bass
import concourse.tile as tile
from concourse import bacc, mybir
from concourse.bass_utils import run_bass_kernel_spmd

BF = ml_dtypes.bfloat16
S, B, T = 1024, 1024, 32
START, END = T - 2, T - 1
NCORES = 8
BC = B // NCORES            # batch per core (128)
NK, EV0, LAG = 16, 4, 6     # renorm cadence / first event / apply lag
CHUNK = 128                 # steps per DMA chunk
P, NGRP, FD = 64, 2, 64     # partitions (tags), tag groups, batch free dim
PB = P + 3                  # backward partitions (+const row, +2 markers)

dt = mybir.dt


def build_program(s_len=S):
    """One SPMD program for all cores: forward + backward half-chains."""
    m = s_len // 2
    chunk = min(CHUNK, m)
    n_ev = (m - EV0 - 1) // NK + 1 if m > EV0 else 0

    nc = bacc.Bacc("TRN2", target_bir_lowering=False, num_devices=NCORES)

    efF_d = nc.dram_tensor("efF", [P, m * FD], dt.bfloat16, kind="ExternalInput")
    efB_d = nc.dram_tensor("efB", [PB, m * FD], dt.bfloat16, kind="ExternalInput")
    y0_d = nc.dram_tensor("y0", [PB, FD], dt.bfloat16, kind="ExternalInput")
    qiF_d = nc.dram_tensor("qinitF", [P, FD], dt.bfloat16, kind="ExternalInput")
    wF_d = nc.dram_tensor("wblkF", [P, P], dt.bfloat16, kind="ExternalInput")
    wB_d = nc.dram_tensor("wblkB", [PB, PB], dt.bfloat16, kind="ExternalInput")
    obF_d = nc.dram_tensor("onesblkF", [P, NGRP], dt.bfloat16, kind="ExternalInput")
    obB_d = nc.dram_tensor("onesblkB", [PB, NGRP], dt.bfloat16, kind="ExternalInput")
    oc_d = nc.dram_tensor("onesbc", [NGRP, P], dt.bfloat16, kind="ExternalInput")

    qF_o = nc.dram_tensor("qF", [P, FD], dt.bfloat16, kind="ExternalOutput")
    qB_o = nc.dram_tensor("qB", [P, FD], dt.bfloat16, kind="ExternalOutput")
    rdF_o = nc.dram_tensor("rdF", [NGRP, max(1, n_ev) * FD], dt.bfloat16,
                           kind="ExternalOutput")
    rdB_o = nc.dram_tensor("rdB", [NGRP, max(1, n_ev) * FD], dt.bfloat16,
                           kind="ExternalOutput")

    with tile.TileContext(nc) as tc:
        with (
            tc.tile_pool(name="singles", bufs=1) as singles,
            tc.tile_pool(name="efpool", bufs=2) as efpool,
            tc.tile_pool(name="small", bufs=2) as small,
            tc.tile_pool(name="ypool", bufs=4) as ypool,
            tc.tile_pool(name="fpool", bufs=4) as fpool,
            tc.tile_pool(name="psF", bufs=3, space="PSUM") as psf_pool,
            tc.tile_pool(name="psB", bufs=3, space="PSUM") as psb_pool,
            tc.tile_pool(name="psE", bufs=1, space="PSUM") as pse_pool,
        ):
            wF_t = singles.tile([P, P], dt.bfloat16, tag="wF", name="wF_t")
            wB_t = singles.tile([PB, PB], dt.bfloat16, tag="wB", name="wB_t")
            obF_t = singles.tile([P, NGRP], dt.bfloat16, tag="obF", name="obF_t")
            obB_t = singles.tile([PB, NGRP], dt.bfloat16, tag="obB", name="obB_t")
            oc_t = singles.tile([NGRP, P], dt.bfloat16, tag="oc", name="oc_t")
            for tl, dr in ((wF_t, wF_d), (wB_t, wB_d), (obF_t, obF_d),
                           (obB_t, obB_d), (oc_t, oc_d)):
                nc.sync.dma_start(out=tl, in_=dr.ap())

            rbF = singles.tile([NGRP, max(1, n_ev) * FD], dt.bfloat16,
                               tag="rbF", name="rbF")
            rbB = singles.tile([NGRP, max(1, n_ev) * FD], dt.bfloat16,
                               tag="rbB", name="rbB")
            f_cur = fpool.tile([P, FD], dt.bfloat16, tag="f", name="f_0")
            nc.sync.dma_start(out=f_cur, in_=qiF_d.ap())

            y_cur = ypool.tile([PB, FD], dt.bfloat16, tag="y", name="y_0")
            nc.sync.dma_start(out=y_cur, in_=y0_d.ap())

            chF = [None, None]
            chB = [None, None]
            pendF, pendB = {}, {}

            def event(i, cur, ob_t, rbuf, pend, is_b):
                # phase 1: colsum + reciprocal now; the broadcast matmul is
                # deferred to the apply step so its semaphore wait never
                # head-of-line-blocks the main matmul stream on PE.
                e = (i - EV0) // NK
                psc = pse_pool.tile([NGRP, FD], dt.float32, tag="psC",
                                    name=f"psC{int(is_b)}_{i}")
                nc.tensor.matmul(psc, ob_t, cur, start=True, stop=True)
                rf = small.tile([NGRP, FD], dt.float32, tag="rf",
                                name=f"rf{int(is_b)}_{i}")
                nc.vector.reciprocal_approx_fast(out=rf, in_=psc)
                rsb = rbuf[:, e * FD:(e + 1) * FD]
                nc.vector.tensor_copy(rsb, rf)
                if i + LAG < (m - 1 if is_b else m):
                    pend[i + LAG] = rsb

            # small leading chunks so the chains start without waiting on
            # a full 1 MB ef transfer; F on the SP HWDGE ring, B on the ACT
            # ring so the two streams don't serialize on one DMA FIFO.
            bounds = [0]
            for inc in (8, 16, 32, 64):
                if bounds[-1] < m:
                    bounds.append(min(m, bounds[-1] + inc))
            while bounds[-1] < m:
                bounds.append(min(m, bounds[-1] + chunk))
            spans = list(zip(bounds[:-1], bounds[1:]))
            for ch, (c_lo, c_hi) in enumerate(spans):
                cw = c_hi - c_lo
                tF = efpool.tile([P, chunk * FD], dt.bfloat16, tag="efF",
                                 name=f"efF_{ch}")
                nc.sync.dma_start(
                    out=tF[:, 0:cw * FD],
                    in_=efF_d.ap()[:, c_lo * FD:c_hi * FD])
                chF[ch % 2] = tF
                tB = efpool.tile([PB, chunk * FD], dt.bfloat16, tag="efB",
                                 name=f"efB_{ch}")
                nc.scalar.dma_start(
                    out=tB[:, 0:cw * FD],
                    in_=efB_d.ap()[:, c_lo * FD:c_hi * FD])
                chB[ch % 2] = tB

                for i in range(c_lo, c_hi):
                    csl = slice((i - c_lo) * FD, (i - c_lo) * FD + FD)
                    # ---------------- forward chain, step i -----------------
                    curF = f_cur
                    if i >= EV0 and (i - EV0) % NK == 0:
                        event(i, curF, obF_t, rbF, pendF, is_b=False)
                    eslF = chF[ch % 2][:, csl]
                    if i in pendF:
                        rsb = pendF.pop(i)
                        psr = pse_pool.tile([P, FD], dt.float32, tag="psR",
                                            name=f"psRF_{i}")
                        nc.tensor.matmul(psr, oc_t, rsb, start=True, stop=True)
                        efx = small.tile([P, FD], dt.bfloat16, tag="efxF",
                                         name=f"efxF_{i}")
                        nc.vector.tensor_mul(efx, psr, eslF)
                        eslF = efx
                    psf = psf_pool.tile([P, FD], dt.float32, tag="psf",
                                        name=f"psf_{i}")
                    nc.tensor.matmul(psf, wF_t, curF, start=True, stop=True)
                    nxtF = fpool.tile([P, FD], dt.bfloat16, tag="f",
                                      name=f"f_{i + 1}")
                    nc.vector.tensor_mul(nxtF, psf, eslF)
                    f_cur = nxtF

                    # ---------------- backward chain, step i ----------------
                    if i >= EV0 and (i - EV0) % NK == 0:
                        event(i, y_cur, obB_t, rbB, pendB, is_b=True)
                    psb = psb_pool.tile([PB, FD], dt.float32, tag="psb",
                                        name=f"psb_{i}")
                    nc.tensor.matmul(psb, wB_t, y_cur, start=True, stop=True)
                    if i < m - 1:
                        eslB = chB[ch % 2][:, csl]
                        if i in pendB:
                            rsb = pendB.pop(i)
                            psr = pse_pool.tile([P, FD], dt.float32, tag="psR",
                                                name=f"psRB_{i}")
                            nc.tensor.matmul(psr, oc_t, rsb, start=True,
                                             stop=True)
                            efx = small.tile([PB, FD], dt.bfloat16, tag="efxB",
                                             name=f"efxB_{i}")
                            nc.vector.tensor_mul(efx[0:P, :], psr, eslB[0:P, :])
                            nc.vector.tensor_copy(efx[P:PB, :], eslB[P:PB, :])
                            eslB = efx
                        y_nxt = ypool.tile([PB, FD], dt.bfloat16, tag="y",
                                           name=f"y_{i + 1}")
                        nc.vector.tensor_mul(y_nxt, psb, eslB)
                        y_cur = y_nxt
                    else:
                        qB_t = singles.tile([P, FD], dt.bfloat16, tag="qBf",
                                            name="qB_t")
                        nc.vector.tensor_copy(qB_t, psb[0:P, :])

            nc.sync.dma_start(out=qF_o.ap(), in_=f_cur)
            nc.sync.dma_start(out=qB_o.ap(), in_=qB_t)
            nc.sync.dma_start(out=rdF_o.ap(), in_=rbF)
            nc.sync.dma_start(out=rdB_o.ap(), in_=rbB)

    nc.finalize()
    return nc


def _host_prep(feats, transition, lengths):
    """Per-core in_maps plus reconstruction metadata."""
    s_len, b_tot = feats.shape[0], feats.shape[1]
    n_cores = b_tot // BC
    m = s_len // 2
    c_pre = feats.max(axis=2)                                # (S, B)
    Ccum = np.vstack([np.zeros((1, b_tot), np.float64),
                      np.cumsum(c_pre.astype(np.float64), 0)])  # (S+1, B)
    ef = np.exp(feats - c_pre[:, :, None]).astype(BF)        # (S, B, T)

    W = np.exp(transition.astype(np.float64))                # [next, prev]
    lhsF = W.T.astype(BF).astype(np.float32)                 # [prev, next]
    lhsB = W.astype(BF).astype(np.float32)                   # [next, prev]
    eT = np.exp(transition[END].astype(np.float64))          # (T,)
    eTb = eT.astype(BF).astype(np.float32)

    wF = np.zeros((P, P), np.float32)
    wB = np.zeros((PB, PB), np.float32)
    for gi in range(NGRP):
        s32 = slice(gi * 32, (gi + 1) * 32)
        wF[s32, s32] = lhsF
        wB[s32, s32] = lhsB
        wB[P + 1 + gi, s32] = eTb                # marker row g -> inject eT
    wB[P, P:PB] = 1.0                            # const row perpetuates
    obF = np.zeros((P, NGRP), np.float32)
    obB = np.zeros((PB, NGRP), np.float32)
    onesbc = np.zeros((NGRP, P), np.float32)
    for gi in range(NGRP):
        obF[gi * 32:(gi + 1) * 32, gi] = 1.0
        obB[gi * 32:(gi + 1) * 32, gi] = 1.0
        onesbc[gi, gi * 32:(gi + 1) * 32] = 1.0
    obB[P, :] = 1.0                              # colsum += 1 (zero-col guard)

    qinitF = np.zeros((P, FD), np.float32)
    qinitF[START, :] = 1.0
    qinitF[32 + START, :] = 1.0

    in_maps = []
    for core in range(n_cores):
        sl = slice(core * BC, (core + 1) * BC)
        A = ef[:, sl, :]                                     # (S, 128, T)
        # brick: [g*32+tag, t, bi] = A[t, g*FD+bi, tag]
        E = (A.reshape(s_len, NGRP, FD, T).transpose(1, 3, 0, 2)
             .reshape(P, s_len, FD)).astype(np.float32)
        EF = np.ascontiguousarray(E[:, :m, :]).reshape(P, m * FD)
        Lc = lengths[sl].astype(int)                         # (128,)
        mark = np.zeros((NGRP, s_len + 1, FD), np.float32)   # [g, t, bi]
        for gi in range(NGRP):
            for bi in range(FD):
                mark[gi, Lc[gi * FD + bi], bi] = 1.0
        # backward stream col i <- t = s_len-2-i, rows: ef, 1, markers at t
        EB = np.zeros((PB, m, FD), np.float32)
        ts = s_len - 2 - np.arange(m)                        # (m,)
        EB[:P] = E[:, ts, :]
        EB[P] = 1.0
        EB[P + 1] = mark[0, ts, :]
        EB[P + 2] = mark[1, ts, :]
        EB = np.ascontiguousarray(EB).reshape(PB, m * FD)
        # y_0: rows = qinitB * ef_{S-1}, const 1, markers at t = S-1
        y0 = np.zeros((PB, FD), np.float32)
        for gi in range(NGRP):
            live = (Lc[gi * FD:(gi + 1) * FD] == s_len).astype(np.float32)
            y0[gi * 32:(gi + 1) * 32, :] = (
                eTb[:, None] * live[None, :] * E[gi * 32:(gi + 1) * 32,
                                                 s_len - 1, :])
        y0[P] = 1.0
        y0[P + 1] = mark[0, s_len - 1, :]
        y0[P + 2] = mark[1, s_len - 1, :]
        in_maps.append({
            "efF": EF.astype(BF),
            "efB": EB.astype(BF),
            "y0": y0.astype(BF),
            "qinitF": qinitF.astype(BF),
            "wblkF": wF.astype(BF),
            "wblkB": wB.astype(BF),
            "onesblkF": obF.astype(BF),
            "onesblkB": obB.astype(BF),
            "onesbc": onesbc.astype(BF),
        })
    return in_maps, Ccum


def _reconstruct(results, Ccum, transition, lengths, s_len=S):
    m = s_len // 2
    n_cores = len(results)
    n_ev = (m - EV0 - 1) // NK + 1 if m > EV0 else 0
    i_apps = EV0 + NK * np.arange(n_ev) + LAG                # (E,)

    out = np.zeros(n_cores * BC, np.float64)
    for core in range(n_cores):
        res = results[core]
        qF = res["qF"].astype(np.float64).reshape(NGRP, 32, FD)
        qB = res["qB"].astype(np.float64).reshape(NGRP, 32, FD)
        lcF = -np.log(np.maximum(
            res["rdF"].astype(np.float64).reshape(NGRP, n_ev, FD), 1e-300))
        lcB = -np.log(np.maximum(
            res["rdB"].astype(np.float64).reshape(NGRP, n_ev, FD), 1e-300))
        for gi in range(NGRP):
            bs = core * BC + gi * FD + np.arange(FD)
            L = lengths[bs]
            dot = (qF[gi] * qB[gi]).sum(axis=0)              # (FD,)
            base = np.log(np.maximum(dot, 1e-300))
            acc = Ccum[L, bs]
            acc = acc + lcF[gi].sum(axis=0)                  # all F events
            i_inj = (s_len - 1) - L                          # -1 when L==s_len
            incB = (i_apps[:, None] >= i_inj[None, :])       # (E, FD)
            acc = acc + (lcB[gi] * incB).sum(axis=0)
            out[bs] = base + acc
    return out


_CACHED_NC = None
LAST_RESULTS = None         # BassKernelResults of the most recent run


def kernel(feats, mask, transition):
    global _CACHED_NC, LAST_RESULTS
    feats = np.asarray(feats, np.float32)
    mask = np.asarray(mask, np.float32)
    transition = np.asarray(transition, np.float32)
    lengths = mask.sum(axis=0).astype(np.int64)              # (B,)

    in_maps, Ccum = _host_prep(feats, transition, lengths)
    if _CACHED_NC is None:
        _CACHED_NC = build_program()
    trace = bool(int(os.environ.get("CRF_TRACE", "0")))
    if trace:
        try:  # supply the NTFF hook module this image's antenv lacks
            import types
            from trn_agent_boot.trn_boot import _ntff_profile_via_ctypes
            if "antenv.axon_hooks" not in sys.modules:
                mm_ = types.ModuleType("antenv.axon_hooks")
                mm_._HOOK = None
                mm_.set_axon_ntff_profile_hook = lambda h: setattr(mm_, "_HOOK", h)
                mm_.get_axon_ntff_profile_hook = lambda: mm_._HOOK
                sys.modules["antenv.axon_hooks"] = mm_
            sys.modules["antenv.axon_hooks"].set_axon_ntff_profile_hook(
                _ntff_profile_via_ctypes("/opt/axon/libaxon_pjrt.so"))
        except Exception as e:  # profiling degrades, run still works
            print(f"ntff hook registration failed: {e}")
    res = run_bass_kernel_spmd(_CACHED_NC, in_maps, core_ids=list(range(NCORES)),
                               trace=trace)
    LAST_RESULTS = res
    out = _reconstruct(res.results, Ccum, transition, lengths)
    return out.astype(np.float32)


if __name__ == "__main__":
    feats = np.load("/tmp/in_feats.npy")
    mask = np.load("/tmp/in_mask.npy")
    trans = np.load("/tmp/in_transition.npy")
    got = kernel(feats, mask, trans)
    exp = np.load("/tmp/expected.npy")
    rel = np.abs(got - exp) / np.maximum(1.0, np.abs(exp))
    print("max rel:", rel.max(), "mean:", rel.mean())



# revision 3
# speedup vs baseline: 3.5225x; 3.5225x over previous
"""CRF forward (partition function) kernel for Trainium2, 8 NeuronCores.

Segmented rank-1 two-pass formulation (exp space), data-parallel over batch:
the per-step operator M_t = diag(ef_t) @ W contracts every direction onto its
Perron image, so the product over a segment of L=16 steps is numerically
rank-1.  Split the S=1024 sequence into K=64 segments; phase 1 runs all 64
segment recurrences in parallel from probe vectors (segment 0 from the true
e_START, the rest from ones), phase 2 re-runs segments 1..63 from the
phase-1 end state of the previous segment (the correct direction up to a
scalar).  The host chains the scalars: gamma_{j+1} = gamma_j * S(yhat_j)/S(y_j)
where S() sums the live rows of a dumped end state.

Per-batch lengths ride in the dead START row: transition[START,:] = -1e4
makes tag START unreachable after step 0, so its state row is repurposed as
an absorbing "done" accumulator.  The stationary matrix row is
exp(trans[END]) (+1 self), and the host-built ef stream switches column b to
the done pattern (e_START) from t = L_b on, which both freezes the chain and
latches Z_b = r^T v_{L_b} into the row.  No extra partitions needed: 4 tag
groups x 32 tags = 128 partitions exactly.

No on-device renorm: the host prescales ef by exp(-log(ef_t . W u*)) (u* =
Perron vector of W), bounding state magnitudes to ~2^43; log-scales are
restored from an fp64 prefix sum.  Layout: 4 chain-streams of 16 segments
fused on the free dim -> every matmul/multiply is (128p x 512f), 64+64 PE
ops and 64+64 DVE ops per phase, ef brick (8.4 MB/core) resident in SBUF so
phase 2 re-reads it without HBM traffic.
"""

import os
import sys

import numpy as np
import ml_dtypes

if "/opt/trn_rl_repo" not in sys.path:
    sys.path.insert(0, "/opt/trn_rl_repo")

import concourse.bass as bass
import concourse.tile as tile
from concourse import bacc, mybir
from concourse.bass_utils import run_bass_kernel_spmd

BF = ml_dtypes.bfloat16
S, B, T = 1024, 1024, 32
START, END = T - 2, T - 1
NCORES = 8
BC = B // NCORES            # batch per core (128)
NG = 4                      # tag-group copies on partitions
FD = BC // NG               # batch free-dim per group (32)
K = 64                      # segments
L = S // K                  # steps per segment (16)
NSTR = 4                    # chain streams
CPS = K // NSTR             # chains per stream (16)
FREE = CPS * FD             # free cols per stream (512)
CSLOT = 4                   # slots per DMA chunk
NCHUNK = L // CSLOT         # chunks per stream (4)

dt = mybir.dt


def build_program():
    nc = bacc.Bacc("TRN2", target_bir_lowering=False, num_devices=NCORES)

    wblk_d = nc.dram_tensor("wblk", [128, 128], dt.bfloat16, kind="ExternalInput")
    init_d = nc.dram_tensor("init", [128, NSTR * FREE], dt.bfloat16,
                            kind="ExternalInput")
    bk_d = [nc.dram_tensor(f"bk{s}", [128, L * FREE], dt.bfloat16,
                           kind="ExternalInput") for s in range(NSTR)]
    yF_o = nc.dram_tensor("yF", [128, K * FD], dt.bfloat16, kind="ExternalOutput")
    yH_o = nc.dram_tensor("yH", [128, K * FD], dt.bfloat16, kind="ExternalOutput")

    rings = [nc.sync, nc.scalar, nc.gpsimd]

    with tile.TileContext(nc) as tc:
        with (
            tc.tile_pool(name="singles", bufs=1) as singles,
            tc.tile_pool(name="st", bufs=3) as st,
            tc.tile_pool(name="ps", bufs=2, space="PSUM") as psp,
        ):
            w_t = singles.tile([128, 128], dt.bfloat16, tag="w", name="w_t")
            nc.sync.dma_start(out=w_t, in_=wblk_d.ap())
            init_t = singles.tile([128, NSTR * FREE], dt.bfloat16, tag="init",
                                  name="init_t")
            nc.scalar.dma_start(out=init_t, in_=init_d.ap())

            # resident ef brick: NSTR x NCHUNK tiles of CSLOT slots each,
            # loaded in slot-progress order round-robin over idle DMA rings
            bk = [[None] * NCHUNK for _ in range(NSTR)]
            ri = 0
            for c in range(NCHUNK):
                for s in range(NSTR):
                    tl = singles.tile([128, CSLOT * FREE], dt.bfloat16,
                                      tag=f"bk{s}_{c}", name=f"bk{s}_{c}")
                    rings[ri % 3].dma_start(
                        out=tl, in_=bk_d[s].ap()[:, c * CSLOT * FREE:
                                                 (c + 1) * CSLOT * FREE])
                    ri += 1
                    bk[s][c] = tl

            # ---------------- phase 1: probe chains ----------------
            cur = [init_t[:, s * FREE:(s + 1) * FREE] for s in range(NSTR)]
            for t in range(L):
                for s in range(NSTR):
                    ps = psp.tile([128, FREE], dt.float32, tag=f"ps{s}",
                                  name=f"ps{s}_{t}")
                    nc.tensor.matmul(ps, w_t, cur[s], start=True, stop=True)
                    nxt = st.tile([128, FREE], dt.bfloat16, tag=f"s{s}",
                                  name=f"s{s}_{t}")
                    csl = (t % CSLOT) * FREE
                    nc.vector.tensor_mul(nxt, ps, bk[s][t // CSLOT][:, csl:csl + FREE])
                    cur[s] = nxt

            for s in range(NSTR):
                rings[s % 3].dma_start(
                    out=yF_o.ap()[:, s * FREE:(s + 1) * FREE], in_=cur[s])

            # phase-2 inits: stream 0 reuses cur[0][0:480] in place (ef
            # offset +32); streams 1..3 assemble [prev-stream tail | own head]
            # via SBUF-to-SBUF DMA on idle rings.
            W2 = FREE - FD      # 480
            ini2 = [cur[0]]
            for s in range(1, NSTR):
                a = singles.tile([128, FREE], dt.bfloat16, tag=f"i2_{s}",
                                 name=f"i2_{s}")
                rings[s % 3].dma_start(out=a[:, 0:FD], in_=cur[s - 1][:, W2:FREE])
                rings[(s + 1) % 3].dma_start(out=a[:, FD:FREE],
                                             in_=cur[s][:, 0:W2])
                ini2.append(a)

            # ---------------- phase 2: true-direction chains ----------------
            wid = [W2] + [FREE] * (NSTR - 1)
            off = [FD] + [0] * (NSTR - 1)
            cur2 = [ini2[s][:, 0:wid[s]] for s in range(NSTR)]
            for t in range(L):
                for s in range(NSTR):
                    ps = psp.tile([128, FREE], dt.float32, tag=f"ps{s}",
                                  name=f"q{s}_{t}")
                    nc.tensor.matmul(ps[:, 0:wid[s]], w_t, cur2[s],
                                     start=True, stop=True)
                    nxt = st.tile([128, FREE], dt.bfloat16, tag=f"h{s}",
                                  name=f"h{s}_{t}")
                    csl = (t % CSLOT) * FREE + off[s]
                    nc.vector.tensor_mul(
                        nxt[:, 0:wid[s]], ps[:, 0:wid[s]],
                        bk[s][t // CSLOT][:, csl:csl + wid[s]])
                    cur2[s] = nxt[:, 0:wid[s]]

            # yH col block j = segment-j end state (block 0 unused)
            nc.sync.dma_start(out=yH_o.ap()[:, FD:FREE], in_=cur2[0])
            for s in range(1, NSTR):
                rings[s % 3].dma_start(
                    out=yH_o.ap()[:, s * FREE:(s + 1) * FREE], in_=cur2[s])

    nc.finalize()
    return nc


def _host_prep(feats, transition, lengths):
    """Builds per-core in_maps + host reconstruction metadata."""
    W = np.exp(transition.astype(np.float64))          # [next, prev]
    r = np.exp(transition[END].astype(np.float64))     # (T,)
    u = np.ones(T)
    for _ in range(100):                               # Perron direction of W
        u = W @ u
        u /= u.sum()
    wu = (W @ u).astype(np.float32)                    # (T,)

    c_pre = feats.max(axis=2)                          # (S, B) f32
    ef0 = np.exp(feats - c_pre[:, :, None])            # (S, B, T) f32
    p = np.log(np.maximum(ef0 @ wu, 1e-30))            # (S, B) f32
    q = (c_pre.astype(np.float64) + p.astype(np.float64))   # (S, B)
    efp = ef0 * np.exp(-p)[:, :, None]                 # (S, B, T)
    del ef0
    # done-pattern: START ef-row is 0 while live, e_START from t >= L_b
    tmask = np.arange(S)[:, None] >= lengths[None, :]  # (S, B)
    efp *= (~tmask)[:, :, None]
    efp[:, :, START] = tmask.astype(np.float32)
    Ps = np.vstack([np.zeros((1, B)), np.cumsum(q, axis=0)])  # (S+1, B)

    # brick: t = (s*CPS + jj)*L + tloc ; b = core*BC + g*FD + f
    # target[core][s][g*32+tag, (tloc*CPS + jj)*FD + f]
    X = efp.reshape(NSTR, CPS, L, NCORES, NG, FD, T)
    X = np.ascontiguousarray(X.transpose(3, 0, 4, 6, 2, 1, 5))
    bricks = X.reshape(NCORES, NSTR, 128, L * FREE).astype(BF)
    del X, efp

    Wt = W.copy()
    Wt[START, :] = r
    Wt[START, START] = 1.0
    lhs = Wt.T.astype(np.float32)                      # [prev, next]
    wblk = np.zeros((128, 128), np.float32)
    for g in range(NG):
        sl = slice(g * T, (g + 1) * T)
        wblk[sl, sl] = lhs
    wblk = wblk.astype(BF)

    init = np.ones((128, NSTR * FREE), np.float32)
    for g in range(NG):
        init[g * T + START, :] = 0.0
    init[:, 0:FD] = 0.0
    for g in range(NG):
        init[g * T + START, 0:FD] = 1.0                # chain 0: e_START
    init = init.astype(BF)

    rt = r.copy()
    rt[START] = 0.0
    in_maps = []
    for core in range(NCORES):
        m = {"wblk": wblk, "init": init}
        for s in range(NSTR):
            m[f"bk{s}"] = np.ascontiguousarray(bricks[core, s])
        in_maps.append(m)
    return in_maps, Ps, rt


def _reconstruct(results, Ps, rt, lengths):
    out = np.zeros(B, np.float64)
    live = np.ones(128, bool)
    for g in range(NG):
        live[g * T + START] = False
    for core in range(NCORES):
        yF = results[core]["yF"].astype(np.float64).reshape(128, K, FD)
        yH = results[core]["yH"].astype(np.float64).reshape(128, K, FD)
        S1 = yF[live].reshape(NG, T - 1, K, FD).sum(axis=1)   # (NG, K, FD)
        S2 = yH[live].reshape(NG, T - 1, K, FD).sum(axis=1)
        dl = np.log(np.maximum(S2, 1e-300)) - np.log(np.maximum(S1, 1e-300))
        # lg[:, j] = sum_{1<=i<j} dl_i  (gamma_1 = 1; segment 0 exact)
        lg = np.zeros((NG, K, FD))
        lg[:, 2:, :] = np.cumsum(dl[:, 1:K - 1, :], axis=1)

        bl = np.arange(BC)
        g, f = bl // FD, bl % FD
        b = core * BC + bl
        Lb = lengths[b]
        j = np.minimum(Lb // L, K - 1).astype(int)
        z = yH[g * T + START, j, f]                           # done-row sample
        full = Lb >= S
        if full.any():
            # r^T (live end state of last segment) per full-length column
            ge, fe = g[full], f[full]
            acc = np.zeros(full.sum())
            for tag in range(T):
                acc += rt[tag] * yH[ge * T + tag, K - 1, fe]
            z = z.copy()
            z[full] = acc
        out[b] = (np.log(np.maximum(z, 1e-300)) + lg[g, j, f]
                  + Ps[np.minimum(Lb, S), b])
    return out


_CACHED_NC = None
LAST_RESULTS = None


def kernel(feats, mask, transition):
    global _CACHED_NC, LAST_RESULTS
    feats = np.asarray(feats, np.float32)
    mask = np.asarray(mask, np.float32)
    transition = np.asarray(transition, np.float32)
    lengths = mask.sum(axis=0).astype(np.int64)

    in_maps, Ps, rt = _host_prep(feats, transition, lengths)
    if _CACHED_NC is None:
        _CACHED_NC = build_program()
    trace = bool(int(os.environ.get("CRF_TRACE", "0")))
    if trace:
        try:  # supply the NTFF hook module this image's antenv lacks
            import types
            from trn_agent_boot.trn_boot import _ntff_profile_via_ctypes
            if "antenv.axon_hooks" not in sys.modules:
                mm_ = types.ModuleType("antenv.axon_hooks")
                mm_._HOOK = None
                mm_.set_axon_ntff_profile_hook = lambda h: setattr(mm_, "_HOOK", h)
                mm_.get_axon_ntff_profile_hook = lambda: mm_._HOOK
                sys.modules["antenv.axon_hooks"] = mm_
            sys.modules["antenv.axon_hooks"].set_axon_ntff_profile_hook(
                _ntff_profile_via_ctypes("/opt/axon/libaxon_pjrt.so"))
        except Exception as e:  # profiling degrades, run still works
            print(f"ntff hook registration failed: {e}")
    res = run_bass_kernel_spmd(_CACHED_NC, in_maps, core_ids=list(range(NCORES)),
                               trace=trace)
    LAST_RESULTS = res
    out = _reconstruct(res.results, Ps, rt, lengths)
    return out.astype(np.float32)


if __name__ == "__main__":
    feats = np.load("/tmp/in_feats.npy")
    mask = np.load("/tmp/in_mask.npy")
    trans = np.load("/tmp/in_transition.npy")
    got = kernel(feats, mask, trans)
    exp = np.load("/tmp/expected.npy")
    rel = np.abs(got - exp) / np.maximum(1.0, np.abs(exp))
    print("max rel:", rel.max(), "mean:", rel.mean())


# revision 4
# speedup vs baseline: 4.2252x; 1.1995x over previous
"""CRF forward (partition function) kernel for Trainium2, 8 NeuronCores.

Warmup-overlap segmented formulation (exp space), data-parallel over batch:
the per-step operator M_t = diag(ef_t) @ W contracts every direction onto
its Perron image, so any chain started from a generic positive vector a few
steps early carries the correct state direction.  Split the S=1024 sequence
into K=32 segments of L=32; chain j starts D=4 steps before its segment
from an all-ones probe (chain 0 from the true e_START, held exactly through
warmup by the done-pattern), runs D+L=36 steps, and is correct-up-to-scale
inside its segment.  All 32 chains are independent end-to-end.  The host
chains the scales: gamma_{j+1} = gamma_j * S(E_j)/S(W_{j+1}), where E_j is
chain j's dumped end state and W_{j+1} is chain j+1's dumped warmup-end
state (both estimates of position c_{j+1}), S() summing live rows in fp64.

Per-batch lengths ride in the dead START row: transition[START,:] = -1e4
makes tag START unreachable after step 0, so its state row is repurposed as
an absorbing "done" accumulator.  The stationary matrix row is
exp(trans[END]) (+1 self), and the host-built ef stream switches column b
to the done pattern (e_START) from t = L_b on, which both freezes the chain
and latches Z_b = r^T v_{L_b} into the row.  4 tag groups x 32 tags = 128
partitions exactly.

No on-device renorm: the host prescales ef by exp(-log(ef_t . W u*)) (u* =
Perron vector of W), bounding state magnitudes to ~2^43; log-scales are
restored from an fp64 prefix sum.  Layout: 2 chain-streams of 16 segments
fused on the free dim -> every matmul/multiply is (128p x 512f); 72 PE ops
and 72 DVE ops total; ef brick (9.4 MB/core) streams through SBUF once.
"""

import os
import sys

import numpy as np
import ml_dtypes

if "/opt/trn_rl_repo" not in sys.path:
    sys.path.insert(0, "/opt/trn_rl_repo")

import concourse.bass as bass
import concourse.tile as tile
from concourse import bacc, mybir
from concourse.bass_utils import run_bass_kernel_spmd

BF = ml_dtypes.bfloat16
S, B, T = 1024, 1024, 32
START, END = T - 2, T - 1
NCORES = 8
BC = B // NCORES            # batch per core (128)
NG = 4                      # tag-group copies on partitions
FD = BC // NG               # batch free-dim per group (32)
K = 32                      # segments
L = S // K                  # steps per segment (32)
D = 4                       # warmup steps per chain
SLOTS = D + L               # 36
NSTR = 2                    # chain streams
CPS = K // NSTR             # chains per stream (16)
FREE = CPS * FD             # free cols per stream (512)
CSLOT = 4                   # slots per DMA chunk
NCHUNK = SLOTS // CSLOT     # chunks per stream (9)

dt = mybir.dt


def build_program():
    nc = bacc.Bacc("TRN2", target_bir_lowering=False, num_devices=NCORES)

    wblk_d = nc.dram_tensor("wblk", [128, 128], dt.bfloat16, kind="ExternalInput")
    init_d = nc.dram_tensor("init", [128, NSTR * FREE], dt.bfloat16,
                            kind="ExternalInput")
    bk_d = [nc.dram_tensor(f"bk{s}", [128, SLOTS * FREE], dt.bfloat16,
                           kind="ExternalInput") for s in range(NSTR)]
    yE_o = nc.dram_tensor("yE", [128, K * FD], dt.bfloat16, kind="ExternalOutput")
    yW_o = nc.dram_tensor("yW", [128, K * FD], dt.bfloat16, kind="ExternalOutput")

    rings = [nc.sync, nc.scalar, nc.gpsimd]

    with tile.TileContext(nc) as tc:
        with (
            tc.tile_pool(name="singles", bufs=1) as singles,
            tc.tile_pool(name="st", bufs=3) as st,
            tc.tile_pool(name="ps", bufs=3, space="PSUM") as psp,
        ):
            w_t = singles.tile([128, 128], dt.bfloat16, tag="w", name="w_t")
            nc.sync.dma_start(out=w_t, in_=wblk_d.ap())
            init_t = singles.tile([128, NSTR * FREE], dt.bfloat16, tag="init",
                                  name="init_t")
            nc.scalar.dma_start(out=init_t, in_=init_d.ap())

            # ef brick: NSTR x NCHUNK tiles of CSLOT slots each, loaded in
            # slot-progress order round-robin over idle DMA rings
            bk = [[None] * NCHUNK for _ in range(NSTR)]
            ri = 0
            for c in range(NCHUNK):
                for s in range(NSTR):
                    tl = singles.tile([128, CSLOT * FREE], dt.bfloat16,
                                      tag=f"bk{s}_{c}", name=f"bk{s}_{c}")
                    rings[ri % 3].dma_start(
                        out=tl, in_=bk_d[s].ap()[:, c * CSLOT * FREE:
                                                 (c + 1) * CSLOT * FREE])
                    ri += 1
                    bk[s][c] = tl

            cur = [init_t[:, s * FREE:(s + 1) * FREE] for s in range(NSTR)]
            for t in range(SLOTS):
                for s in range(NSTR):
                    ps = psp.tile([128, FREE], dt.float32, tag=f"ps{s}",
                                  name=f"ps{s}_{t}")
                    nc.tensor.matmul(ps, w_t, cur[s], start=True, stop=True)
                    nxt = st.tile([128, FREE], dt.bfloat16, tag=f"s{s}",
                                  name=f"s{s}_{t}")
                    csl = (t % CSLOT) * FREE
                    nc.vector.tensor_mul(nxt, ps, bk[s][t // CSLOT][:, csl:csl + FREE])
                    cur[s] = nxt
                if t == D - 1:           # dump warmup-end states
                    for s in range(NSTR):
                        rings[s].dma_start(
                            out=yW_o.ap()[:, s * FREE:(s + 1) * FREE],
                            in_=cur[s])

            for s in range(NSTR):
                rings[s].dma_start(
                    out=yE_o.ap()[:, s * FREE:(s + 1) * FREE], in_=cur[s])

    nc.finalize()
    return nc


def _host_prep(feats, transition, lengths):
    """Builds per-core in_maps + host reconstruction metadata."""
    W = np.exp(transition.astype(np.float64))          # [next, prev]
    r = np.exp(transition[END].astype(np.float64))     # (T,)
    u = np.ones(T)
    for _ in range(100):                               # Perron direction of W
        u = W @ u
        u /= u.sum()
    wu = (W @ u).astype(np.float32)                    # (T,)

    c_pre = feats.max(axis=2)                          # (S, B) f32
    ef0 = np.exp(feats - c_pre[:, :, None])            # (S, B, T) f32
    p = np.log(np.maximum(ef0 @ wu, 1e-30))            # (S, B) f32
    q = (c_pre.astype(np.float64) + p.astype(np.float64))   # (S, B)
    efp = ef0 * np.exp(-p)[:, :, None]                 # (S, B, T)
    del ef0
    # done-pattern: START ef-row is 0 while live, e_START from t >= L_b
    tmask = np.arange(S)[:, None] >= lengths[None, :]  # (S, B)
    efp *= (~tmask)[:, :, None]
    efp[:, :, START] = tmask.astype(np.float32)
    Ps = np.vstack([np.zeros((1, B)), np.cumsum(q, axis=0)])  # (S+1, B)

    # prepend D done-pattern rows so chain-0 warmup (pos<0) holds e_START
    pad = np.zeros((D, B, T), np.float32)
    pad[:, :, START] = 1.0
    effx = np.concatenate([pad, efp], axis=0).astype(BF)     # (S+D, B, T)
    del efp

    # brick: chain j=s*CPS+jj, slot tl -> pos = j*L - D + tl
    # target[core][s][g*32+tag, (tl*CPS + jj)*FD + f]
    bricks = np.empty((NCORES, NSTR, 128, SLOTS * FREE), dtype=BF)
    jj_idx = np.arange(CPS)
    tl_idx = np.arange(SLOTS)
    for s in range(NSTR):
        j = s * CPS + jj_idx                           # (CPS,)
        pos = (j[:, None] * L - D + tl_idx[None, :]) + D    # (CPS, SLOTS) >= 0
        Xs = effx[pos]                                 # (CPS, SLOTS, B, T)
        Xs = Xs.reshape(CPS, SLOTS, NCORES, NG, FD, T)
        Xs = Xs.transpose(2, 3, 5, 1, 0, 4)            # [core, g, tag, tl, jj, f]
        bricks[:, s] = np.ascontiguousarray(Xs).reshape(NCORES, 128, SLOTS * FREE)

    Wt = W.copy()
    Wt[START, :] = r
    Wt[START, START] = 1.0
    lhs = Wt.T.astype(np.float32)                      # [prev, next]
    wblk = np.zeros((128, 128), np.float32)
    for g in range(NG):
        sl = slice(g * T, (g + 1) * T)
        wblk[sl, sl] = lhs
    wblk = wblk.astype(BF)

    init = np.ones((128, NSTR * FREE), np.float32)
    for g in range(NG):
        init[g * T + START, :] = 0.0
    init[:, 0:FD] = 0.0
    for g in range(NG):
        init[g * T + START, 0:FD] = 1.0                # chain 0: e_START
    init = init.astype(BF)

    rt = r.copy()
    rt[START] = 0.0
    in_maps = []
    for core in range(NCORES):
        m = {"wblk": wblk, "init": init}
        for s in range(NSTR):
            m[f"bk{s}"] = np.ascontiguousarray(bricks[core, s])
        in_maps.append(m)
    return in_maps, Ps, rt


def _reconstruct(results, Ps, rt, lengths):
    out = np.zeros(B, np.float64)
    live = np.ones(128, bool)
    for g in range(NG):
        live[g * T + START] = False
    for core in range(NCORES):
        # col block j*FD..(j+1)*FD = chain j  (j = s*CPS + jj)
        yE = results[core]["yE"].astype(np.float64).reshape(128, K, FD)
        yW = results[core]["yW"].astype(np.float64).reshape(128, K, FD)
        SE = yE[live].reshape(NG, T - 1, K, FD).sum(axis=1)   # (NG, K, FD)
        SW = yW[live].reshape(NG, T - 1, K, FD).sum(axis=1)
        # lg[:, j] = sum_{0<=i<j} (ln SE_i - ln SW_{i+1})
        dl = (np.log(np.maximum(SE[:, :K - 1, :], 1e-300))
              - np.log(np.maximum(SW[:, 1:, :], 1e-300)))     # (NG, K-1, FD)
        lg = np.zeros((NG, K, FD))
        lg[:, 1:, :] = np.cumsum(dl, axis=1)

        bl = np.arange(BC)
        g, f = bl // FD, bl % FD
        b = core * BC + bl
        Lb = lengths[b]
        j = np.minimum(Lb // L, K - 1).astype(int)
        z = yE[g * T + START, j, f]                           # done-row sample
        full = Lb >= S
        if full.any():
            # r^T (live end state of last chain) per full-length column
            ge, fe = g[full], f[full]
            acc = np.zeros(full.sum())
            for tag in range(T):
                acc += rt[tag] * yE[ge * T + tag, K - 1, fe]
            z = z.copy()
            z[full] = acc
        out[b] = (np.log(np.maximum(z, 1e-300)) + lg[g, j, f]
                  + Ps[np.minimum(Lb, S), b])
    return out


_CACHED_NC = None
LAST_RESULTS = None


def kernel(feats, mask, transition):
    global _CACHED_NC, LAST_RESULTS
    feats = np.asarray(feats, np.float32)
    mask = np.asarray(mask, np.float32)
    transition = np.asarray(transition, np.float32)
    lengths = mask.sum(axis=0).astype(np.int64)

    in_maps, Ps, rt = _host_prep(feats, transition, lengths)
    if _CACHED_NC is None:
        _CACHED_NC = build_program()
    trace = bool(int(os.environ.get("CRF_TRACE", "0")))
    if trace:
        try:  # supply the NTFF hook module this image's antenv lacks
            import types
            from trn_agent_boot.trn_boot import _ntff_profile_via_ctypes
            if "antenv.axon_hooks" not in sys.modules:
                mm_ = types.ModuleType("antenv.axon_hooks")
                mm_._HOOK = None
                mm_.set_axon_ntff_profile_hook = lambda h: setattr(mm_, "_HOOK", h)
                mm_.get_axon_ntff_profile_hook = lambda: mm_._HOOK
                sys.modules["antenv.axon_hooks"] = mm_
            sys.modules["antenv.axon_hooks"].set_axon_ntff_profile_hook(
                _ntff_profile_via_ctypes("/opt/axon/libaxon_pjrt.so"))
        except Exception as e:  # profiling degrades, run still works
            print(f"ntff hook registration failed: {e}")
    res = run_bass_kernel_spmd(_CACHED_NC, in_maps, core_ids=list(range(NCORES)),
                               trace=trace)
    LAST_RESULTS = res
    out = _reconstruct(res.results, Ps, rt, lengths)
    return out.astype(np.float32)


if __name__ == "__main__":
    feats = np.load("/tmp/in_feats.npy")
    mask = np.load("/tmp/in_mask.npy")
    trans = np.load("/tmp/in_transition.npy")
    got = kernel(feats, mask, trans)
    exp = np.load("/tmp/expected.npy")
    rel = np.abs(got - exp) / np.maximum(1.0, np.abs(exp))
    print("max rel:", rel.max(), "mean:", rel.mean())
